# revision 7
# baseline (speedup 1.0000x reference)
"""EuclideanCodebook (VQ) forward + EMA update on 8 Trainium2 NeuronCores.

Strategy (data-parallel over flattened N, codebook replicated):
  per core (N_loc = 8192 rows of x):
    scores = 2*x@embed.T - |embed|^2  via PE matmuls (fp32r or fp16-split),
        with the -|e|^2/2 bias folded in as K=1 matmul rows
    argmax via DVE max8/max_index; onehot = (iota == idx) on GpSimd (fp16)
    quantize = embed[idx] via indirect-DMA gather
    embed_sum = onehot.T @ x  (+ counts via ones column) accumulated on PE
    AllReduce(embed_sum ++ counts) across the 8 cores
    EMA + laplace-smoothed normalization on-device (every core, identical)
  host: shard/stack + final reshapes only.
"""

import numpy as np
import ml_dtypes
import concourse.bass as bass
import concourse.bacc as bacc
import concourse.tile as tile
import concourse.mybir as mybir
from concourse import bass_utils
from concourse.bass import ts, ds

# ---- problem constants (hardcoded from the spec) ----
N_CORES = 8
B0, B1 = 16, 4096
N = B0 * B1            # 65536
D = 512
K = 1024
NLOC = N // N_CORES    # 8192
P = 128
TILES = NLOC // P      # 64
GROUP = 8              # tiles per group (x/xT DMA granularity)
GROUPS = TILES // GROUP
SPAN_TILES = 16        # tiles per embed_sum PSUM accumulation span
SPANS = TILES // SPAN_TILES
DCH = D // P           # 4 contraction chunks for scores
MCH = K // P           # 8 code chunks for embed_sum
DECAY = 0.99
EPS = 1e-5

# scores matmul mode: "fp32r" (fast, hw-reduced-precision fp32)
# or "fp16x3" (3-term fp16 hi/lo split, ~fp32 accuracy at 3x PE cost)
SCORES_MODE = "fp16x3"
# mantissa bits (explicit) assumed exactly-representable by the scores matmul
# input format; used to split the -|e|^2/2 bias into hi+lo rows.
BIAS_HI_MASK = np.uint32(0xFFFF0000)  # bf16-safe default; refined after probe

dt = mybir.dt

_CACHE = {}


def _build(mode):
    nc = bacc.Bacc("TRN2", target_bir_lowering=False, debug=False,
                   num_devices=N_CORES)

    # ---- I/O ----
    if mode == "fp32r":
        xT = nc.dram_tensor("xT", [D, NLOC], dt.float32r, kind="ExternalInput")
        embT = nc.dram_tensor("embT", [D, K], dt.float32r, kind="ExternalInput")
        bias2 = nc.dram_tensor("bias2", [2, K], dt.float32r, kind="ExternalInput")
        sdty = dt.float32r
    else:
        xTh = nc.dram_tensor("xTh", [D, NLOC], dt.float16, kind="ExternalInput")
        xTl = nc.dram_tensor("xTl", [D, NLOC], dt.float16, kind="ExternalInput")
        embTh = nc.dram_tensor("embTh", [D, K], dt.float16, kind="ExternalInput")
        embTl = nc.dram_tensor("embTl", [D, K], dt.float16, kind="ExternalInput")
        bias2 = nc.dram_tensor("bias2", [2, K], dt.float16, kind="ExternalInput")
        sdty = dt.float16
    x16 = nc.dram_tensor("x16", [NLOC, D], dt.float16, kind="ExternalInput")
    embed = nc.dram_tensor("embed", [K, D], dt.float32, kind="ExternalInput")
    cs_in = nc.dram_tensor("cs_in", [K], dt.float32, kind="ExternalInput")
    ea_in = nc.dram_tensor("ea_in", [K, D], dt.float32, kind="ExternalInput")

    quant = nc.dram_tensor("quant", [NLOC, D], dt.float32, kind="ExternalOutput")
    ind_o = nc.dram_tensor("ind_o", [NLOC], dt.uint32, kind="ExternalOutput")
    ncs_o = nc.dram_tensor("ncs_o", [K], dt.float32, kind="ExternalOutput")
    nea_o = nc.dram_tensor("nea_o", [K, D], dt.float32, kind="ExternalOutput")
    ne_o = nc.dram_tensor("ne_o", [K, D], dt.float32, kind="ExternalOutput")

    iota_np = np.tile(np.arange(K, dtype=np.float16), (P, 1))
    iota_const = nc.inline_tensor(iota_np, name="iota16")

    with tile.TileContext(nc) as tc:
        with (
            tc.tile_pool(name="const", bufs=1) as cpool,
            tc.tile_pool(name="xt", bufs=2) as xt_pool,
            tc.tile_pool(name="x16p", bufs=3) as x16_pool,
            tc.tile_pool(name="oh", bufs=SPAN_TILES + 8) as oh_pool,
            tc.tile_pool(name="small", bufs=6) as sm_pool,
            tc.tile_pool(name="gq", bufs=4) as gq_pool,
            tc.tile_pool(name="psc", bufs=2, space="PSUM") as psc,
            tc.tile_pool(name="pes", bufs=2, space="PSUM") as pes,
            tc.tile_pool(name="pcnt", bufs=1, space="PSUM") as pcnt,
            tc.tile_pool(name="dram", bufs=1, space="DRAM") as dpool,
        ):
            # ---- constants ----
            if mode == "fp32r":
                embT_sb = cpool.tile([P, DCH, K], dt.float32r, tag="embT")
                nc.sync.dma_start(
                    embT_sb[:], embT[:].rearrange("(c p) k -> p c k", p=P))
            else:
                embTh_sb = cpool.tile([P, DCH, K], dt.float16, tag="embTh")
                embTl_sb = cpool.tile([P, DCH, K], dt.float16, tag="embTl")
                nc.sync.dma_start(
                    embTh_sb[:], embTh[:].rearrange("(c p) k -> p c k", p=P))
                nc.sync.dma_start(
                    embTl_sb[:], embTl[:].rearrange("(c p) k -> p c k", p=P))
            bias_hi = cpool.tile([1, K], sdty, tag="biash")
            bias_lo = cpool.tile([1, K], sdty, tag="biasl")
            nc.sync.dma_start(bias_hi[:], bias2[0:1, :])
            nc.sync.dma_start(bias_lo[:], bias2[1:2, :])
            ones1 = cpool.tile([1, P], sdty, tag="ones1")
            nc.vector.memset(ones1[:], 1.0)
            onesd = cpool.tile([P, 1], dt.float16, tag="onesd")
            nc.vector.memset(onesd[:], 1.0)
            iota_sb = cpool.tile([P, K], dt.float16, tag="iota")
            nc.sync.dma_start(iota_sb[:], iota_const.ap())
            ind_sb = cpool.tile([P, TILES], dt.uint32, tag="ind")
            es_acc = cpool.tile([P, MCH, D], dt.float32, tag="esacc")

            cnt_ps = pcnt.tile([P, MCH], dt.float32, tag="cnt")

            oh_tiles = [None] * TILES
            x16_g = [None] * GROUPS
            xt_g = [None] * GROUPS

            def emit_es_span(s):
                for m in range(MCH):
                    es_ps = pes.tile([P, D], dt.float32, tag="es")
                    for j in range(SPAN_TILES):
                        t = s * SPAN_TILES + j
                        g, tt = t // GROUP, t % GROUP
                        oh_m = oh_tiles[t][:, ts(m, P)]
                        nc.tensor.matmul(
                            es_ps[:], lhsT=oh_m, rhs=x16_g[g][:, tt, :],
                            start=(j == 0), stop=(j == SPAN_TILES - 1))
                        # start=True clears the whole PSUM bank, and all 8
                        # count columns share one bank — so clear exactly once
                        # (first matmul overall); later first-touches of other
                        # columns overwrite via has_written=0.
                        nc.tensor.matmul(
                            cnt_ps[:, m:m + 1], lhsT=oh_m, rhs=onesd[:],
                            start=(m == 0 and s == 0 and j == 0),
                            stop=(m == MCH - 1 and s == SPANS - 1
                                  and j == SPAN_TILES - 1),
                            skip_group_check=True)
                    if s == 0:
                        nc.vector.tensor_copy(es_acc[:, m, :], es_ps[:])
                    else:
                        nc.vector.tensor_add(
                            out=es_acc[:, m, :], in0=es_acc[:, m, :], in1=es_ps[:])

            for t in range(TILES):
                g, tt = t // GROUP, t % GROUP
                if tt == 0:
                    if mode == "fp32r":
                        xg = xt_pool.tile([P, DCH, GROUP * P], dt.float32r,
                                          tag="xtg")
                        nc.sync.dma_start(
                            xg[:],
                            xT[:, ds(g * GROUP * P, GROUP * P)].rearrange(
                                "(c p) n -> p c n", p=P))
                        xt_g[g] = xg
                    else:
                        xgh = xt_pool.tile([P, DCH, GROUP * P], dt.float16,
                                           tag="xtgh")
                        xgl = xt_pool.tile([P, DCH, GROUP * P], dt.float16,
                                           tag="xtgl")
                        nc.sync.dma_start(
                            xgh[:],
                            xTh[:, ds(g * GROUP * P, GROUP * P)].rearrange(
                                "(c p) n -> p c n", p=P))
                        nc.sync.dma_start(
                            xgl[:],
                            xTl[:, ds(g * GROUP * P, GROUP * P)].rearrange(
                                "(c p) n -> p c n", p=P))
                        xt_g[g] = (xgh, xgl)
                    x16t = x16_pool.tile([P, GROUP, D], dt.float16, tag="x16g")
                    nc.sync.dma_start(
                        x16t[:],
                        x16[ds(g * GROUP * P, GROUP * P), :].rearrange(
                            "(a p) d -> p a d", p=P))
                    x16_g[g] = x16t

                # ---- scores ----
                sc = psc.tile([P, 2, 512], dt.float32, tag="sc")
                for kc in range(2):
                    ksl = ds(512 * kc, 512)
                    if mode == "fp32r":
                        for dc in range(DCH):
                            nc.tensor.matmul(
                                sc[:, kc, :],
                                lhsT=xt_g[g][:, dc, ts(tt, P)],
                                rhs=embT_sb[:, dc, ksl],
                                start=(dc == 0), stop=False)
                    else:
                        xgh, xgl = xt_g[g]
                        for i, (xx, ee) in enumerate(
                                ((xgh, embTh_sb), (xgh, embTl_sb),
                                 (xgl, embTh_sb))):
                            for dc in range(DCH):
                                nc.tensor.matmul(
                                    sc[:, kc, :],
                                    lhsT=xx[:, dc, ts(tt, P)],
                                    rhs=ee[:, dc, ksl],
                                    start=(i == 0 and dc == 0), stop=False)
                    nc.tensor.matmul(sc[:, kc, :], lhsT=ones1[:],
                                     rhs=bias_hi[:, ksl],
                                     start=False, stop=False)
                    nc.tensor.matmul(sc[:, kc, :], lhsT=ones1[:],
                                     rhs=bias_lo[:, ksl],
                                     start=False, stop=True)

                sc_flat = sc[:].rearrange("p a b -> p (a b)")
                m8 = sm_pool.tile([P, 8], dt.float32, tag="m8")
                i8 = sm_pool.tile([P, 8], dt.uint32, tag="i8")
                nc.vector.max(out=m8[:], in_=sc_flat)
                nc.vector.max_index(out=i8[:], in_max=m8[:], in_values=sc_flat)
                idxf = sm_pool.tile([P, 1], dt.float32, tag="idxf")
                nc.vector.tensor_copy(idxf[:], i8[:, 0:1])
                nc.vector.tensor_copy(ind_sb[:, t:t + 1], i8[:, 0:1])

                oh = oh_pool.tile([P, K], dt.float16, tag="oh")
                nc.gpsimd.tensor_scalar(
                    oh[:], iota_sb[:], idxf[:], None, mybir.AluOpType.is_equal)
                oh_tiles[t] = oh

                gq = gq_pool.tile([P, D], dt.float32, tag="gq")
                nc.gpsimd.indirect_dma_start(
                    out=gq[:], out_offset=None, in_=embed[:],
                    in_offset=bass.IndirectOffsetOnAxis(ap=i8[:, 0:1], axis=0))
                nc.sync.dma_start(quant[ds(t * P, P), :], gq[:])

                if t % SPAN_TILES == SPAN_TILES - 1:
                    emit_es_span(t // SPAN_TILES)

            # ---- indices out ----
            nc.sync.dma_start(ind_o[:].rearrange("(t p) -> p t", p=P), ind_sb[:])

            # ---- all-reduce embed_sum ++ counts ----
            cnt_sb = cpool.tile([P, MCH], dt.float32, tag="cntsb")
            nc.vector.tensor_copy(cnt_sb[:], cnt_ps[:])
            cc_in = dpool.tile([P, MCH, D + 1], dt.float32, tag="ccin")
            cc_out = dpool.tile([P, MCH, D + 1], dt.float32, tag="ccout")
            nc.sync.dma_start(cc_in[:, :, 0:D], es_acc[:])
            nc.sync.dma_start(cc_in[:, :, D], cnt_sb[:])
            nc.gpsimd.collective_compute(
                "AllReduce", mybir.AluOpType.add,
                replica_groups=[list(range(N_CORES))],
                ins=[cc_in[:].opt()], outs=[cc_out[:].opt()])
            nc.sync.dma_start(es_acc[:], cc_out[:, :, 0:D])
            nc.sync.dma_start(cnt_sb[:], cc_out[:, :, D])

            # ---- EMA tail (identical on every core) ----
            cs_sb = cpool.tile([P, MCH], dt.float32, tag="cssb")
            nc.sync.dma_start(cs_sb[:], cs_in[:].rearrange("(m p) -> p m", p=P))
            ea_sb = cpool.tile([P, MCH, D], dt.float32, tag="easb")
            nc.sync.dma_start(ea_sb[:],
                              ea_in[:].rearrange("(m p) d -> p m d", p=P))

            ncs = cpool.tile([P, MCH], dt.float32, tag="ncs")
            tmp1 = cpool.tile([P, MCH], dt.float32, tag="tmp1")
            nc.vector.tensor_scalar_mul(tmp1[:], cnt_sb[:], 1.0 - DECAY)
            nc.vector.tensor_scalar_mul(ncs[:], cs_sb[:], DECAY)
            nc.vector.tensor_add(out=ncs[:], in0=ncs[:], in1=tmp1[:])
            nc.sync.dma_start(ncs_o[:].rearrange("(m p) -> p m", p=P), ncs[:])

            rowsum = cpool.tile([P, 1], dt.float32, tag="rowsum")
            nc.vector.tensor_reduce(rowsum[:], ncs[:],
                                    axis=mybir.AxisListType.X,
                                    op=mybir.AluOpType.add)
            ones128 = cpool.tile([P, 1], dt.float32, tag="ones128")
            nc.vector.memset(ones128[:], 1.0)
            ones1f = cpool.tile([1, P], dt.float32, tag="ones1f")
            nc.vector.memset(ones1f[:], 1.0)
            nps = pes.tile([P, 4], dt.float32, tag="es")
            nc.tensor.matmul(nps[0:1, 0:1], lhsT=rowsum[:], rhs=ones128[:],
                             start=True, stop=True)
            nsb = cpool.tile([1, 1], dt.float32, tag="nsb")
            nc.vector.tensor_copy(nsb[:], nps[0:1, 0:1])
            nbc_ps = pes.tile([P, 4], dt.float32, tag="es")
            nc.tensor.matmul(nbc_ps[:, 0:1], lhsT=ones1f[:], rhs=nsb[:],
                             start=True, stop=True)
            nbc = cpool.tile([P, 1], dt.float32, tag="nbc")
            nc.vector.tensor_copy(nbc[:], nbc_ps[:, 0:1])

            denom = cpool.tile([P, 1], dt.float32, tag="denom")
            nc.vector.tensor_scalar_add(denom[:], nbc[:], float(K) * EPS)
            rden = cpool.tile([P, 1], dt.float32, tag="rden")
            nc.vector.reciprocal(rden[:], denom[:])
            factor = cpool.tile([P, 1], dt.float32, tag="factor")
            nc.vector.tensor_mul(out=factor[:], in0=rden[:], in1=nbc[:])
            csm = cpool.tile([P, MCH], dt.float32, tag="csm")
            nc.vector.tensor_scalar_add(csm[:], ncs[:], EPS)
            cs2 = cpool.tile([P, MCH], dt.float32, tag="cs2")
            nc.vector.tensor_scalar(cs2[:], csm[:], factor[:], None,
                                    mybir.AluOpType.mult)
            invcs = cpool.tile([P, MCH], dt.float32, tag="invcs")
            nc.vector.reciprocal(invcs[:], cs2[:])

            # EMA in place: es_acc <- 0.01*es_acc ; ea_sb <- 0.99*ea_sb + es_acc
            nc.vector.tensor_scalar_mul(es_acc[:], es_acc[:], 1.0 - DECAY)
            nc.vector.tensor_scalar_mul(ea_sb[:], ea_sb[:], DECAY)
            nc.vector.tensor_add(out=ea_sb[:], in0=ea_sb[:], in1=es_acc[:])
            nc.sync.dma_start(nea_o[:].rearrange("(m p) d -> p m d", p=P),
                              ea_sb[:])
            # new_embed = nea / cs, reusing es_acc as scratch
            for m in range(MCH):
                nc.vector.tensor_scalar(es_acc[:, m, :], ea_sb[:, m, :],
                                        invcs[:, m:m + 1], None,
                                        mybir.AluOpType.mult)
            nc.sync.dma_start(ne_o[:].rearrange("(m p) d -> p m d", p=P),
                              es_acc[:])

    nc.compile()
    return nc


def _get_nc(mode=None):
    mode = mode or SCORES_MODE
    if mode not in _CACHE:
        _CACHE[mode] = _build(mode)
    return _CACHE[mode]


def _split_hi_lo_f32(v64, mask):
    hi = v64.astype(np.float32)
    hi = (hi.view(np.uint32) & mask).view(np.float32)
    lo = (v64 - hi.astype(np.float64)).astype(np.float32)
    return hi, lo


def _prep_inputs(x, embed, cluster_size, embed_avg, mode):
    xf = np.ascontiguousarray(np.asarray(x, dtype=np.float32).reshape(N, D))
    embed = np.ascontiguousarray(np.asarray(embed, dtype=np.float32))
    cluster_size = np.ascontiguousarray(
        np.asarray(cluster_size, dtype=np.float32))
    embed_avg = np.ascontiguousarray(np.asarray(embed_avg, dtype=np.float32))

    q64 = (embed.astype(np.float64) ** 2).sum(axis=1)
    b64 = -q64 / 2.0
    if mode == "fp32r":
        bh, bl = _split_hi_lo_f32(b64, BIAS_HI_MASK)
        bias2 = np.stack([bh, bl]).astype(np.float32)
        embT = np.ascontiguousarray(embed.T)
    else:
        bh = b64.astype(np.float16)
        bl = (b64 - bh.astype(np.float64)).astype(np.float16)
        bias2 = np.stack([bh, bl]).astype(np.float16)
        eT = np.ascontiguousarray(embed.T)
        embTh = eT.astype(np.float16)
        embTl = (eT - embTh.astype(np.float32)).astype(np.float16)

    in_maps = []
    for r in range(N_CORES):
        sl = slice(r * NLOC, (r + 1) * NLOC)
        xr = xf[sl]
        m = {
            "x16": xr.astype(np.float16),
            "embed": embed,
            "cs_in": cluster_size,
            "ea_in": embed_avg,
            "bias2": bias2,
        }
        xrT = np.ascontiguousarray(xr.T)
        if mode == "fp32r":
            m["xT"] = xrT
            m["embT"] = embT
        else:
            xh = xrT.astype(np.float16)
            m["xTh"] = xh
            m["xTl"] = (xrT - xh.astype(np.float32)).astype(np.float16)
            m["embTh"] = embTh
            m["embTl"] = embTl
        in_maps.append(m)
    return in_maps


def _assemble(results):
    quant = np.concatenate([r["quant"] for r in results], axis=0)
    quantize = quant.reshape(B0, B1, D)
    ind = np.concatenate([r["ind_o"] for r in results]).view(np.int32)
    embed_ind = ind.reshape(B0, B1)
    r0 = results[0]
    ncs = r0["ncs_o"]
    nea = r0["nea_o"]
    ne = r0["ne_o"]
    return quantize, embed_ind, ncs, nea, ne


def _run_full(inputs, mode=None, trace=False, **run_kwargs):
    mode = mode or SCORES_MODE
    nc = _get_nc(mode)
    in_maps = _prep_inputs(inputs["x"], inputs["embed"],
                           inputs["cluster_size"], inputs["embed_avg"], mode)
    res = bass_utils.run_bass_kernel_spmd(
        nc, in_maps, core_ids=list(range(N_CORES)), trace=trace, **run_kwargs)
    return _assemble(res.results), res


def kernel(x, embed, cluster_size, embed_avg):
    out, _ = _run_full(dict(x=x, embed=embed, cluster_size=cluster_size,
                            embed_avg=embed_avg))
    return out


# revision 16
# speedup vs baseline: 595.0193x; 595.0193x over previous
"""EuclideanCodebook (VQ) forward + EMA update on 8 Trainium2 NeuronCores.

Strategy (data-parallel over flattened N, codebook replicated):
  per core (N_loc = 8192 rows of x):
    scores = 2*x@embed.T - |embed|^2  via PE matmuls (fp32r or fp16-split),
        with the -|e|^2/2 bias folded in as K=1 matmul rows
    argmax via DVE max8/max_index; onehot = (iota == idx) on GpSimd (fp16)
    quantize = embed[idx] via indirect-DMA gather
    embed_sum = onehot.T @ x  (+ counts via ones column) accumulated on PE
    AllReduce(embed_sum ++ counts) across the 8 cores
    EMA + laplace-smoothed normalization on-device (every core, identical)
  host: shard/stack + final reshapes only.
"""

import numpy as np
import concourse.bass as bass
import concourse.bacc as bacc
import concourse.tile as tile
import concourse.mybir as mybir
from concourse import bass_utils
from concourse.bass import ts, ds

# ---- problem constants (hardcoded from the spec) ----
N_CORES = 8
B0, B1 = 16, 4096
N = B0 * B1            # 65536
D = 512
K = 1024
NLOC = N // N_CORES    # 8192
P = 128
TILES = NLOC // P      # 64
GROUP = 8              # tiles per group (x/xT DMA granularity)
GROUPS = TILES // GROUP
SPAN_TILES = 16        # tiles per embed_sum PSUM accumulation span
SPANS = TILES // SPAN_TILES
DCH = D // P           # 4 contraction chunks for scores
MCH = K // P           # 8 code chunks for embed_sum
DECAY = 0.99
EPS = 1e-5

# scores matmul mode: "fp32r" (fast, hw-reduced-precision fp32)
# or "fp16x3" (3-term fp16 hi/lo split, ~fp32 accuracy at 3x PE cost)
SCORES_MODE = "fp16x3"
# mantissa bits (explicit) assumed exactly-representable by the scores matmul
# input format; used to split the -|e|^2/2 bias into hi+lo rows.
BIAS_HI_MASK = np.uint32(0xFFFF0000)  # bf16-safe default; refined after probe

dt = mybir.dt

_CACHE = {}


def _build(mode):
    nc = bacc.Bacc("TRN2", target_bir_lowering=False, debug=False,
                   num_devices=N_CORES)

    # ---- I/O ----
    if mode == "fp32r":
        xT = nc.dram_tensor("xT", [D, NLOC], dt.float32r, kind="ExternalInput")
        embT = nc.dram_tensor("embT", [D, K], dt.float32r, kind="ExternalInput")
        bias2 = nc.dram_tensor("bias2", [2, K], dt.float32r, kind="ExternalInput")
        sdty = dt.float32r
    else:
        xTh = nc.dram_tensor("xTh", [D, NLOC], dt.float16, kind="ExternalInput")
        xTl = nc.dram_tensor("xTl", [D, NLOC], dt.float16, kind="ExternalInput")
        embTh = nc.dram_tensor("embTh", [D, K], dt.float16, kind="ExternalInput")
        embTl = nc.dram_tensor("embTl", [D, K], dt.float16, kind="ExternalInput")
        bias2 = nc.dram_tensor("bias2", [2, K], dt.float16, kind="ExternalInput")
        sdty = dt.float16
    x16 = nc.dram_tensor("x16", [NLOC, D], dt.float16, kind="ExternalInput")
    embed = nc.dram_tensor("embed", [K, D], dt.float32, kind="ExternalInput")
    cs_in = nc.dram_tensor("cs_in", [K], dt.float32, kind="ExternalInput")
    ea_in = nc.dram_tensor("ea_in", [K, D], dt.float32, kind="ExternalInput")

    quant = nc.dram_tensor("quant", [NLOC, D], dt.float32, kind="ExternalOutput")
    ind_o = nc.dram_tensor("ind_o", [NLOC], dt.uint32, kind="ExternalOutput")
    ncs_o = nc.dram_tensor("ncs_o", [K], dt.float32, kind="ExternalOutput")
    nea_o = nc.dram_tensor("nea_o", [K, D], dt.float32, kind="ExternalOutput")
    ne_o = nc.dram_tensor("ne_o", [K, D], dt.float32, kind="ExternalOutput")

    iota_np = np.tile(np.arange(K, dtype=np.float16), (P, 1))
    iota_const = nc.inline_tensor(iota_np, name="iota16")

    with tile.TileContext(nc) as tc:
        with (
            tc.tile_pool(name="const", bufs=1) as cpool,
            tc.tile_pool(name="xt", bufs=2) as xt_pool,
            tc.tile_pool(name="x16p", bufs=3) as x16_pool,
            tc.tile_pool(name="oh", bufs=SPAN_TILES + 8) as oh_pool,
            tc.tile_pool(name="small", bufs=6) as sm_pool,
            tc.tile_pool(name="gq", bufs=4) as gq_pool,
            tc.tile_pool(name="psc", bufs=2, space="PSUM") as psc,
            tc.tile_pool(name="pes", bufs=2, space="PSUM") as pes,
            tc.tile_pool(name="pcnt", bufs=1, space="PSUM") as pcnt,
            tc.tile_pool(name="dram", bufs=1, space="DRAM") as dpool,
        ):
            # ---- constants ----
            if mode == "fp32r":
                embT_sb = cpool.tile([P, DCH, K], dt.float32r, tag="embT")
                nc.sync.dma_start(
                    embT_sb[:], embT[:].rearrange("(c p) k -> p c k", p=P))
            else:
                embTh_sb = cpool.tile([P, DCH, K], dt.float16, tag="embTh")
                embTl_sb = cpool.tile([P, DCH, K], dt.float16, tag="embTl")
                for c in range(DCH):
                    nc.sync.dma_start(
                        embTh_sb[:, c, :],
                        embTh[ds(c * P, P), :])
                    nc.sync.dma_start(
                        embTl_sb[:, c, :],
                        embTl[ds(c * P, P), :])
            bias_hi = cpool.tile([1, K], sdty, tag="biash")
            bias_lo = cpool.tile([1, K], sdty, tag="biasl")
            nc.sync.dma_start(bias_hi[:], bias2[0:1, :])
            nc.sync.dma_start(bias_lo[:], bias2[1:2, :])
            ones1 = cpool.tile([1, P], sdty, tag="ones1")
            nc.vector.memset(ones1[:], 1.0)
            onesd = cpool.tile([P, 1], dt.float16, tag="onesd")
            nc.vector.memset(onesd[:], 1.0)
            iota_sb = cpool.tile([P, K], dt.float16, tag="iota")
            nc.sync.dma_start(iota_sb[:], iota_const.ap())
            ind_sb = cpool.tile([P, TILES], dt.uint32, tag="ind")
            es_acc = cpool.tile([P, MCH, D], dt.float32, tag="esacc")

            cnt_ps = pcnt.tile([P, MCH], dt.float32, tag="cnt")

            oh_tiles = [None] * TILES
            x16_g = [None] * GROUPS
            xt_g = [None] * GROUPS

            def emit_es_span(s):
                for m in range(MCH):
                    es_ps = pes.tile([P, D], dt.float32, tag="es")
                    for j in range(SPAN_TILES):
                        t = s * SPAN_TILES + j
                        g, tt = t // GROUP, t % GROUP
                        oh_m = oh_tiles[t][:, ts(m, P)]
                        nc.tensor.matmul(
                            es_ps[:], lhsT=oh_m, rhs=x16_g[g][:, tt, :],
                            start=(j == 0), stop=(j == SPAN_TILES - 1))
                        # start=True clears the whole PSUM bank, and all 8
                        # count columns share one bank — so clear exactly once
                        # (first matmul overall); later first-touches of other
                        # columns overwrite via has_written=0.
                        nc.tensor.matmul(
                            cnt_ps[:, m:m + 1], lhsT=oh_m, rhs=onesd[:],
                            start=(m == 0 and s == 0 and j == 0),
                            stop=(m == MCH - 1 and s == SPANS - 1
                                  and j == SPAN_TILES - 1),
                            skip_group_check=True)
                    if s == 0:
                        nc.vector.tensor_copy(es_acc[:, m, :], es_ps[:])
                    else:
                        nc.vector.tensor_add(
                            out=es_acc[:, m, :], in0=es_acc[:, m, :], in1=es_ps[:])

            for t in range(TILES):
                g, tt = t // GROUP, t % GROUP
                if tt == 0:
                    if mode == "fp32r":
                        xg = xt_pool.tile([P, DCH, GROUP * P], dt.float32r,
                                          tag="xtg")
                        nc.sync.dma_start(
                            xg[:],
                            xT[:, ds(g * GROUP * P, GROUP * P)].rearrange(
                                "(c p) n -> p c n", p=P))
                        xt_g[g] = xg
                    else:
                        xgh = xt_pool.tile([P, DCH, GROUP * P], dt.float16,
                                           tag="xtgh")
                        xgl = xt_pool.tile([P, DCH, GROUP * P], dt.float16,
                                           tag="xtgl")
                        nsl = ds(g * GROUP * P, GROUP * P)
                        for c in range(DCH):
                            nc.sync.dma_start(
                                xgh[:, c, :], xTh[ds(c * P, P), nsl])
                            nc.sync.dma_start(
                                xgl[:, c, :], xTl[ds(c * P, P), nsl])
                        xt_g[g] = (xgh, xgl)
                    x16t = x16_pool.tile([P, GROUP, D], dt.float16, tag="x16g")
                    nc.sync.dma_start(
                        x16t[:],
                        x16[ds(g * GROUP * P, GROUP * P), :].rearrange(
                            "(a p) d -> p a d", p=P))
                    x16_g[g] = x16t

                # ---- scores ----
                # kc innermost: one weight load serves both k-chunk matmuls
                # (the two kc psum regions are different banks, so their
                # start=True bank-clears don't interfere)
                sc = psc.tile([P, 2, 512], dt.float32, tag="sc")
                if mode == "fp32r":
                    terms = ((xt_g[g], embT_sb),)
                else:
                    xgh, xgl = xt_g[g]
                    terms = ((xgh, embTh_sb), (xgh, embTl_sb), (xgl, embTh_sb))
                for i, (xx, ee) in enumerate(terms):
                    for dc in range(DCH):
                        for kc in range(2):
                            nc.tensor.matmul(
                                sc[:, kc, :],
                                lhsT=xx[:, dc, ts(tt, P)],
                                rhs=ee[:, dc, ds(512 * kc, 512)],
                                start=(i == 0 and dc == 0), stop=False)
                for kc in range(2):
                    ksl = ds(512 * kc, 512)
                    nc.tensor.matmul(sc[:, kc, :], lhsT=ones1[:],
                                     rhs=bias_hi[:, ksl],
                                     start=False, stop=False)
                    nc.tensor.matmul(sc[:, kc, :], lhsT=ones1[:],
                                     rhs=bias_lo[:, ksl],
                                     start=False, stop=True)

                sc_flat = sc[:].rearrange("p a b -> p (a b)")
                m8 = sm_pool.tile([P, 8], dt.float32, tag="m8")
                i8 = sm_pool.tile([P, 8], dt.uint32, tag="i8")
                nc.vector.max(out=m8[:], in_=sc_flat)
                nc.vector.max_index(out=i8[:], in_max=m8[:], in_values=sc_flat)
                idxf = sm_pool.tile([P, 1], dt.float32, tag="idxf")
                nc.vector.tensor_copy(idxf[:], i8[:, 0:1])
                nc.vector.tensor_copy(ind_sb[:, t:t + 1], i8[:, 0:1])

                oh = oh_pool.tile([P, K], dt.float16, tag="oh")
                nc.gpsimd.tensor_scalar(
                    oh[:], iota_sb[:], idxf[:], None, mybir.AluOpType.is_equal)
                oh_tiles[t] = oh

                gq = gq_pool.tile([P, D], dt.float32, tag="gq")
                nc.gpsimd.indirect_dma_start(
                    out=gq[:], out_offset=None, in_=embed[:],
                    in_offset=bass.IndirectOffsetOnAxis(ap=i8[:, 0:1], axis=0))
                nc.sync.dma_start(quant[ds(t * P, P), :], gq[:])

                if t == TILES // 2:
                    # prescale EMA inputs mid-kernel so the scheduler can
                    # overlap them with compute (tail shrink)
                    cs_sb = cpool.tile([P, MCH], dt.float32, tag="cssb")
                    nc.sync.dma_start(
                        cs_sb[:], cs_in[:].rearrange("(m p) -> p m", p=P))
                    ncs_pre = cpool.tile([P, MCH], dt.float32, tag="ncspre")
                    nc.vector.tensor_scalar_mul(ncs_pre[:], cs_sb[:], DECAY)
                    ea_sb = cpool.tile([P, MCH, D], dt.float32, tag="easb")
                    nc.sync.dma_start(
                        ea_sb[:], ea_in[:].rearrange("(m p) d -> p m d", p=P))
                    nc.vector.tensor_scalar_mul(ea_sb[:], ea_sb[:], DECAY)

                if t % SPAN_TILES == SPAN_TILES - 1:
                    emit_es_span(t // SPAN_TILES)

            # ---- indices out ----
            nc.sync.dma_start(ind_o[:].rearrange("(t p) -> p t", p=P), ind_sb[:])

            # ---- all-reduce embed_sum ++ counts ----
            cnt_sb = cpool.tile([P, MCH], dt.float32, tag="cntsb")
            nc.vector.tensor_copy(cnt_sb[:], cnt_ps[:])
            cc_in = dpool.tile([P, MCH, D + 1], dt.float32, tag="ccin")
            cc_out = dpool.tile([P, MCH, D + 1], dt.float32, tag="ccout",
                                addr_space="Shared")
            nc.sync.dma_start(cc_in[:, :, 0:D], es_acc[:])
            nc.sync.dma_start(cc_in[:, :, D], cnt_sb[:])
            nc.gpsimd.collective_compute(
                "AllReduce", mybir.AluOpType.add,
                replica_groups=[list(range(N_CORES))],
                ins=[cc_in[:].opt()], outs=[cc_out[:].opt()])
            nc.sync.dma_start(es_acc[:], cc_out[:, :, 0:D])
            nc.sync.dma_start(cnt_sb[:], cc_out[:, :, D])

            # ---- EMA tail (identical on every core) ----
            ncs = cpool.tile([P, MCH], dt.float32, tag="ncs")
            tmp1 = cpool.tile([P, MCH], dt.float32, tag="tmp1")
            nc.vector.tensor_scalar_mul(tmp1[:], cnt_sb[:], 1.0 - DECAY)
            nc.vector.tensor_add(out=ncs[:], in0=ncs_pre[:], in1=tmp1[:])
            nc.sync.dma_start(ncs_o[:].rearrange("(m p) -> p m", p=P), ncs[:])

            rowsum = cpool.tile([P, 1], dt.float32, tag="rowsum")
            nc.vector.tensor_reduce(rowsum[:], ncs[:],
                                    axis=mybir.AxisListType.X,
                                    op=mybir.AluOpType.add)
            ones128 = cpool.tile([P, 1], dt.float32, tag="ones128")
            nc.vector.memset(ones128[:], 1.0)
            ones1f = cpool.tile([1, P], dt.float32, tag="ones1f")
            nc.vector.memset(ones1f[:], 1.0)
            nps = pes.tile([P, 4], dt.float32, tag="es")
            nc.tensor.matmul(nps[0:1, 0:1], lhsT=rowsum[:], rhs=ones128[:],
                             start=True, stop=True)
            nsb = cpool.tile([1, 1], dt.float32, tag="nsb")
            nc.vector.tensor_copy(nsb[:], nps[0:1, 0:1])
            nbc_ps = pes.tile([P, 4], dt.float32, tag="es")
            nc.tensor.matmul(nbc_ps[:, 0:1], lhsT=ones1f[:], rhs=nsb[:],
                             start=True, stop=True)
            nbc = cpool.tile([P, 1], dt.float32, tag="nbc")
            nc.vector.tensor_copy(nbc[:], nbc_ps[:, 0:1])

            denom = cpool.tile([P, 1], dt.float32, tag="denom")
            nc.vector.tensor_scalar_add(denom[:], nbc[:], float(K) * EPS)
            rden = cpool.tile([P, 1], dt.float32, tag="rden")
            nc.vector.reciprocal(rden[:], denom[:])
            factor = cpool.tile([P, 1], dt.float32, tag="factor")
            nc.vector.tensor_mul(out=factor[:], in0=rden[:], in1=nbc[:])
            csm = cpool.tile([P, MCH], dt.float32, tag="csm")
            nc.vector.tensor_scalar_add(csm[:], ncs[:], EPS)
            cs2 = cpool.tile([P, MCH], dt.float32, tag="cs2")
            nc.vector.tensor_scalar(cs2[:], csm[:], factor[:], None,
                                    mybir.AluOpType.mult)
            invcs = cpool.tile([P, MCH], dt.float32, tag="invcs")
            nc.vector.reciprocal(invcs[:], cs2[:])

            # EMA per code-chunk, pipelining compute with output DMAs:
            # es_acc <- 0.01*es_acc ; ea_sb (pre-scaled 0.99) += es_acc = nea
            # ne = nea * (1/cs), reusing es_acc as scratch
            nea_dst = nea_o[:].rearrange("(m p) d -> p m d", p=P)
            ne_dst = ne_o[:].rearrange("(m p) d -> p m d", p=P)
            for m in range(MCH):
                nc.vector.tensor_scalar_mul(es_acc[:, m, :], es_acc[:, m, :],
                                            1.0 - DECAY)
                nc.vector.tensor_add(out=ea_sb[:, m, :], in0=ea_sb[:, m, :],
                                     in1=es_acc[:, m, :])
                nc.sync.dma_start(nea_dst[:, m, :], ea_sb[:, m, :])
                nc.vector.tensor_scalar(es_acc[:, m, :], ea_sb[:, m, :],
                                        invcs[:, m:m + 1], None,
                                        mybir.AluOpType.mult)
                nc.sync.dma_start(ne_dst[:, m, :], es_acc[:, m, :])

    nc.compile()
    return nc


def _get_nc(mode=None):
    mode = mode or SCORES_MODE
    if mode not in _CACHE:
        _CACHE[mode] = _build(mode)
    return _CACHE[mode]


def _split_hi_lo_f32(v64, mask):
    hi = v64.astype(np.float32)
    hi = (hi.view(np.uint32) & mask).view(np.float32)
    lo = (v64 - hi.astype(np.float64)).astype(np.float32)
    return hi, lo


def _prep_inputs(x, embed, cluster_size, embed_avg, mode):
    xf = np.ascontiguousarray(np.asarray(x, dtype=np.float32).reshape(N, D))
    embed = np.ascontiguousarray(np.asarray(embed, dtype=np.float32))
    cluster_size = np.ascontiguousarray(
        np.asarray(cluster_size, dtype=np.float32))
    embed_avg = np.ascontiguousarray(np.asarray(embed_avg, dtype=np.float32))

    q64 = (embed.astype(np.float64) ** 2).sum(axis=1)
    b64 = -q64 / 2.0
    if mode == "fp32r":
        bh, bl = _split_hi_lo_f32(b64, BIAS_HI_MASK)
        bias2 = np.stack([bh, bl]).astype(np.float32)
        embT = np.ascontiguousarray(embed.T)
    else:
        bh = b64.astype(np.float16)
        bl = (b64 - bh.astype(np.float64)).astype(np.float16)
        bias2 = np.stack([bh, bl]).astype(np.float16)
        eT = np.ascontiguousarray(embed.T)
        embTh = eT.astype(np.float16)
        embTl = (eT - embTh.astype(np.float32)).astype(np.float16)

    in_maps = []
    for r in range(N_CORES):
        sl = slice(r * NLOC, (r + 1) * NLOC)
        xr = xf[sl]
        m = {
            "x16": xr.astype(np.float16),
            "embed": embed,
            "cs_in": cluster_size,
            "ea_in": embed_avg,
            "bias2": bias2,
        }
        xrT = np.ascontiguousarray(xr.T)
        if mode == "fp32r":
            m["xT"] = xrT
            m["embT"] = embT
        else:
            xh = xrT.astype(np.float16)
            m["xTh"] = xh
            m["xTl"] = (xrT - xh.astype(np.float32)).astype(np.float16)
            m["embTh"] = embTh
            m["embTl"] = embTl
        in_maps.append(m)
    return in_maps


def _assemble(results):
    quant = np.concatenate([r["quant"] for r in results], axis=0)
    quantize = quant.reshape(B0, B1, D)
    ind = np.concatenate([r["ind_o"] for r in results]).view(np.int32)
    embed_ind = ind.reshape(B0, B1)
    r0 = results[0]
    ncs = r0["ncs_o"]
    nea = r0["nea_o"]
    ne = r0["ne_o"]
    return quantize, embed_ind, ncs, nea, ne


def _run_full(inputs, mode=None, trace=False, **run_kwargs):
    mode = mode or SCORES_MODE
    nc = _get_nc(mode)
    in_maps = _prep_inputs(inputs["x"], inputs["embed"],
                           inputs["cluster_size"], inputs["embed_avg"], mode)
    res = bass_utils.run_bass_kernel_spmd(
        nc, in_maps, core_ids=list(range(N_CORES)), trace=trace, **run_kwargs)
    return _assemble(res.results), res


def kernel(x, embed, cluster_size, embed_avg):
    out, _ = _run_full(dict(x=x, embed=embed, cluster_size=cluster_size,
                            embed_avg=embed_avg))
    return out


# revision 23
# speedup vs baseline: 647.6706x; 1.0885x over previous
"""EuclideanCodebook (VQ) forward + EMA update on 8 Trainium2 NeuronCores.

Strategy (data-parallel over flattened N, codebook replicated):
  per core (N_loc = 8192 rows of x):
    scores = 2*x@embed.T - |embed|^2  via PE matmuls (fp32r or fp16-split),
        with the -|e|^2/2 bias folded in as K=1 matmul rows
    argmax via DVE max8/max_index; onehot = (iota == idx) on GpSimd (fp16)
    quantize = embed[idx] via indirect-DMA gather
    embed_sum = onehot.T @ x  (+ counts via ones column) accumulated on PE
    AllReduce(embed_sum ++ counts) across the 8 cores
    EMA + laplace-smoothed normalization on-device (every core, identical)
  host: shard/stack + final reshapes only.
"""

import numpy as np
import concourse.bass as bass
import concourse.bacc as bacc
import concourse.tile as tile
import concourse.mybir as mybir
from concourse import bass_utils
from concourse.bass import ts, ds

# ---- problem constants (hardcoded from the spec) ----
N_CORES = 8
B0, B1 = 16, 4096
N = B0 * B1            # 65536
D = 512
K = 1024
NLOC = N // N_CORES    # 8192
P = 128
TILES = NLOC // P      # 64
GROUP = 8              # tiles per group (x/xT DMA granularity)
GROUPS = TILES // GROUP
SPAN_TILES = 16        # tiles per embed_sum PSUM accumulation span
SPANS = TILES // SPAN_TILES
DCH = D // P           # 4 contraction chunks for scores
MCH = K // P           # 8 code chunks for embed_sum
DECAY = 0.99
EPS = 1e-5

# scores matmul mode: "fp32r" (fast, hw-reduced-precision fp32)
# or "fp16x3" (3-term fp16 hi/lo split, ~fp32 accuracy at 3x PE cost)
SCORES_MODE = "fp16x3"
# mantissa bits (explicit) assumed exactly-representable by the scores matmul
# input format; used to split the -|e|^2/2 bias into hi+lo rows.
BIAS_HI_MASK = np.uint32(0xFFFF0000)  # bf16-safe default; refined after probe

dt = mybir.dt

_CACHE = {}


def _build(mode):
    nc = bacc.Bacc("TRN2", target_bir_lowering=False, debug=False,
                   num_devices=N_CORES)

    # ---- I/O ----
    if mode == "fp32r":
        xT = nc.dram_tensor("xT", [D, NLOC], dt.float32r, kind="ExternalInput")
        embT = nc.dram_tensor("embT", [D, K], dt.float32r, kind="ExternalInput")
        bias2 = nc.dram_tensor("bias2", [2, K], dt.float32r, kind="ExternalInput")
        sdty = dt.float32r
    else:
        xTh = nc.dram_tensor("xTh", [D, NLOC], dt.float16, kind="ExternalInput")
        xTl = nc.dram_tensor("xTl", [D, NLOC], dt.float16, kind="ExternalInput")
        embTh = nc.dram_tensor("embTh", [D, K], dt.float16, kind="ExternalInput")
        embTl = nc.dram_tensor("embTl", [D, K], dt.float16, kind="ExternalInput")
        # |e|^2/2 per code, replicated across partitions (subtracted from the
        # matmul result on DVE — cheaper than K=1 bias matmuls on PE)
        q2rep = nc.dram_tensor("q2rep", [P, K], dt.float32, kind="ExternalInput")
        sdty = dt.float16
    x16 = nc.dram_tensor("x16", [NLOC, D], dt.float16, kind="ExternalInput")
    embed = nc.dram_tensor("embed", [K, D], dt.float32, kind="ExternalInput")
    cs_in = nc.dram_tensor("cs_in", [K], dt.float32, kind="ExternalInput")
    ea_in = nc.dram_tensor("ea_in", [K, D], dt.float32, kind="ExternalInput")

    quant = nc.dram_tensor("quant", [NLOC, D], dt.float32, kind="ExternalOutput")
    ind_o = nc.dram_tensor("ind_o", [NLOC], dt.uint32, kind="ExternalOutput")
    ncs_o = nc.dram_tensor("ncs_o", [K], dt.float32, kind="ExternalOutput")
    nea_o = nc.dram_tensor("nea_o", [K, D], dt.float32, kind="ExternalOutput")
    ne_o = nc.dram_tensor("ne_o", [K, D], dt.float32, kind="ExternalOutput")

    iota_np = np.tile(np.arange(K, dtype=np.float16), (P, 1))
    iota_const = nc.inline_tensor(iota_np, name="iota16")

    with tile.TileContext(nc) as tc:
        with (
            tc.tile_pool(name="const", bufs=1) as cpool,
            tc.tile_pool(name="xt", bufs=2) as xt_pool,
            tc.tile_pool(name="x16p", bufs=3) as x16_pool,
            tc.tile_pool(name="oh", bufs=SPAN_TILES + 8) as oh_pool,
            tc.tile_pool(name="small", bufs=6) as sm_pool,
            tc.tile_pool(name="gq", bufs=4) as gq_pool,
            tc.tile_pool(name="psc", bufs=2, space="PSUM") as psc,
            tc.tile_pool(name="pes", bufs=2, space="PSUM") as pes,
            tc.tile_pool(name="pcnt", bufs=1, space="PSUM") as pcnt,
            tc.tile_pool(name="dram", bufs=1, space="DRAM") as dpool,
        ):
            # ---- constants ----
            if mode == "fp32r":
                embT_sb = cpool.tile([P, DCH, K], dt.float32r, tag="embT")
                nc.sync.dma_start(
                    embT_sb[:], embT[:].rearrange("(c p) k -> p c k", p=P))
            else:
                embTh_sb = cpool.tile([P, DCH, K], dt.float16, tag="embTh")
                embTl_sb = cpool.tile([P, DCH, K], dt.float16, tag="embTl")
                for c in range(DCH):
                    nc.sync.dma_start(
                        embTh_sb[:, c, :],
                        embTh[ds(c * P, P), :])
                    nc.sync.dma_start(
                        embTl_sb[:, c, :],
                        embTl[ds(c * P, P), :])
            if mode == "fp32r":
                bias_hi = cpool.tile([1, K], sdty, tag="biash")
                bias_lo = cpool.tile([1, K], sdty, tag="biasl")
                nc.sync.dma_start(bias_hi[:], bias2[0:1, :])
                nc.sync.dma_start(bias_lo[:], bias2[1:2, :])
                ones1 = cpool.tile([1, P], sdty, tag="ones1")
                nc.vector.memset(ones1[:], 1.0)
            else:
                q2_sb = cpool.tile([P, K], dt.float32, tag="q2")
                nc.sync.dma_start(q2_sb[:], q2rep[:])
            onesd = cpool.tile([P, 1], dt.float16, tag="onesd")
            nc.vector.memset(onesd[:], 1.0)
            iota_sb = cpool.tile([P, K], dt.float16, tag="iota")
            nc.sync.dma_start(iota_sb[:], iota_const.ap())
            ind_sb = cpool.tile([P, TILES], dt.uint32, tag="ind")
            es_acc = cpool.tile([P, MCH, D], dt.float32, tag="esacc")

            cnt_ps = pcnt.tile([P, MCH], dt.float32, tag="cnt")

            oh_tiles = [None] * TILES
            x16_g = [None] * GROUPS
            xt_g = [None] * GROUPS

            def emit_es_span(s):
                for m in range(MCH):
                    es_ps = pes.tile([P, D], dt.float32, tag="es")
                    for j in range(SPAN_TILES):
                        t = s * SPAN_TILES + j
                        g, tt = t // GROUP, t % GROUP
                        oh_m = oh_tiles[t][:, ts(m, P)]
                        nc.tensor.matmul(
                            es_ps[:], lhsT=oh_m, rhs=x16_g[g][:, tt, :],
                            start=(j == 0), stop=(j == SPAN_TILES - 1))
                        # start=True clears the whole PSUM bank, and all 8
                        # count columns share one bank — so clear exactly once
                        # (first matmul overall); later first-touches of other
                        # columns overwrite via has_written=0.
                        nc.tensor.matmul(
                            cnt_ps[:, m:m + 1], lhsT=oh_m, rhs=onesd[:],
                            start=(m == 0 and s == 0 and j == 0),
                            stop=(m == MCH - 1 and s == SPANS - 1
                                  and j == SPAN_TILES - 1),
                            skip_group_check=True)
                    if s == 0:
                        nc.vector.tensor_copy(es_acc[:, m, :], es_ps[:])
                    else:
                        nc.vector.tensor_add(
                            out=es_acc[:, m, :], in0=es_acc[:, m, :], in1=es_ps[:])

            for t in range(TILES):
                g, tt = t // GROUP, t % GROUP
                if tt == 0:
                    if mode == "fp32r":
                        xg = xt_pool.tile([P, DCH, GROUP * P], dt.float32r,
                                          tag="xtg")
                        nc.sync.dma_start(
                            xg[:],
                            xT[:, ds(g * GROUP * P, GROUP * P)].rearrange(
                                "(c p) n -> p c n", p=P))
                        xt_g[g] = xg
                    else:
                        xgh = xt_pool.tile([P, DCH, GROUP * P], dt.float16,
                                           tag="xtgh")
                        xgl = xt_pool.tile([P, DCH, GROUP * P], dt.float16,
                                           tag="xtgl")
                        nsl = ds(g * GROUP * P, GROUP * P)
                        for c in range(DCH):
                            nc.sync.dma_start(
                                xgh[:, c, :], xTh[ds(c * P, P), nsl])
                            nc.sync.dma_start(
                                xgl[:, c, :], xTl[ds(c * P, P), nsl])
                        xt_g[g] = (xgh, xgl)
                    x16t = x16_pool.tile([P, GROUP, D], dt.float16, tag="x16g")
                    nc.sync.dma_start(
                        x16t[:],
                        x16[ds(g * GROUP * P, GROUP * P), :].rearrange(
                            "(a p) d -> p a d", p=P))
                    x16_g[g] = x16t

                # ---- scores ----
                # kc innermost: one weight load serves both k-chunk matmuls
                # (the two kc psum regions are different banks, so their
                # start=True bank-clears don't interfere)
                sc = psc.tile([P, 2, 512], dt.float32, tag="sc")
                if mode == "fp32r":
                    terms = ((xt_g[g], embT_sb),)
                else:
                    xgh, xgl = xt_g[g]
                    terms = ((xgh, embTh_sb), (xgh, embTl_sb), (xgl, embTh_sb))
                nterms = len(terms)
                for i, (xx, ee) in enumerate(terms):
                    for dc in range(DCH):
                        for kc in range(2):
                            nc.tensor.matmul(
                                sc[:, kc, :],
                                lhsT=xx[:, dc, ts(tt, P)],
                                rhs=ee[:, dc, ds(512 * kc, 512)],
                                start=(i == 0 and dc == 0),
                                stop=(mode != "fp32r" and i == nterms - 1
                                      and dc == DCH - 1))
                if mode == "fp32r":
                    for kc in range(2):
                        ksl = ds(512 * kc, 512)
                        nc.tensor.matmul(sc[:, kc, :], lhsT=ones1[:],
                                         rhs=bias_hi[:, ksl],
                                         start=False, stop=False,
                                         skip_group_check=True)
                        nc.tensor.matmul(sc[:, kc, :], lhsT=ones1[:],
                                         rhs=bias_lo[:, ksl],
                                         start=False, stop=True,
                                         skip_group_check=True)

                sc_flat = sc[:].rearrange("p a b -> p (a b)")
                if mode != "fp32r":
                    # scores -= |e|^2/2, in place in PSUM on the DVE
                    nc.vector.tensor_tensor(
                        out=sc_flat, in0=sc_flat, in1=q2_sb[:],
                        op=mybir.AluOpType.subtract)
                m8 = sm_pool.tile([P, 8], dt.float32, tag="m8")
                i8 = sm_pool.tile([P, 8], dt.uint32, tag="i8")
                nc.vector.max(out=m8[:], in_=sc_flat)
                nc.vector.max_index(out=i8[:], in_max=m8[:], in_values=sc_flat)
                idxf = sm_pool.tile([P, 1], dt.float32, tag="idxf")
                nc.vector.tensor_copy(idxf[:], i8[:, 0:1])
                nc.vector.tensor_copy(ind_sb[:, t:t + 1], i8[:, 0:1])

                oh = oh_pool.tile([P, K], dt.float16, tag="oh")
                nc.gpsimd.tensor_scalar(
                    oh[:], iota_sb[:], idxf[:], None, mybir.AluOpType.is_equal)
                oh_tiles[t] = oh

                gq = gq_pool.tile([P, D], dt.float32, tag="gq")
                nc.gpsimd.indirect_dma_start(
                    out=gq[:], out_offset=None, in_=embed[:],
                    in_offset=bass.IndirectOffsetOnAxis(ap=i8[:, 0:1], axis=0))
                nc.sync.dma_start(quant[ds(t * P, P), :], gq[:])

                if t == TILES // 2:
                    # prescale EMA inputs mid-kernel so the scheduler can
                    # overlap them with compute (tail shrink)
                    cs_sb = cpool.tile([P, MCH], dt.float32, tag="cssb")
                    nc.sync.dma_start(
                        cs_sb[:], cs_in[:].rearrange("(m p) -> p m", p=P))
                    ncs_pre = cpool.tile([P, MCH], dt.float32, tag="ncspre")
                    nc.vector.tensor_scalar_mul(ncs_pre[:], cs_sb[:], DECAY)
                    ea_sb = cpool.tile([P, MCH, D], dt.float32, tag="easb")
                    nc.sync.dma_start(
                        ea_sb[:], ea_in[:].rearrange("(m p) d -> p m d", p=P))
                    nc.vector.tensor_scalar_mul(ea_sb[:], ea_sb[:], DECAY)

                # emit each embed_sum span 2 tiles into the NEXT span so the
                # PE has fresh score matmuls queued while the span's last
                # onehot tiles finish on GpSimd (removes a per-span bubble)
                if t >= SPAN_TILES and t % SPAN_TILES == 1:
                    emit_es_span(t // SPAN_TILES - 1)

            emit_es_span(SPANS - 1)

            # ---- indices out ----
            nc.sync.dma_start(ind_o[:].rearrange("(t p) -> p t", p=P), ind_sb[:])

            # ---- all-reduce embed_sum ++ counts ----
            cnt_sb = cpool.tile([P, MCH], dt.float32, tag="cntsb")
            nc.vector.tensor_copy(cnt_sb[:], cnt_ps[:])
            cc_in = dpool.tile([P, MCH, D + 1], dt.float32, tag="ccin")
            cc_out = dpool.tile([P, MCH, D + 1], dt.float32, tag="ccout",
                                addr_space="Shared")
            nc.sync.dma_start(cc_in[:, :, 0:D], es_acc[:])
            nc.sync.dma_start(cc_in[:, :, D], cnt_sb[:])
            nc.gpsimd.collective_compute(
                "AllReduce", mybir.AluOpType.add,
                replica_groups=[list(range(N_CORES))],
                ins=[cc_in[:].opt()], outs=[cc_out[:].opt()])
            nc.sync.dma_start(es_acc[:], cc_out[:, :, 0:D])
            nc.sync.dma_start(cnt_sb[:], cc_out[:, :, D])

            # ---- EMA tail (identical on every core) ----
            ncs = cpool.tile([P, MCH], dt.float32, tag="ncs")
            tmp1 = cpool.tile([P, MCH], dt.float32, tag="tmp1")
            nc.vector.tensor_scalar_mul(tmp1[:], cnt_sb[:], 1.0 - DECAY)
            nc.vector.tensor_add(out=ncs[:], in0=ncs_pre[:], in1=tmp1[:])
            nc.sync.dma_start(ncs_o[:].rearrange("(m p) -> p m", p=P), ncs[:])

            rowsum = cpool.tile([P, 1], dt.float32, tag="rowsum")
            nc.vector.tensor_reduce(rowsum[:], ncs[:],
                                    axis=mybir.AxisListType.X,
                                    op=mybir.AluOpType.add)
            ones128 = cpool.tile([P, 1], dt.float32, tag="ones128")
            nc.vector.memset(ones128[:], 1.0)
            ones1f = cpool.tile([1, P], dt.float32, tag="ones1f")
            nc.vector.memset(ones1f[:], 1.0)
            nps = pes.tile([P, 4], dt.float32, tag="es")
            nc.tensor.matmul(nps[0:1, 0:1], lhsT=rowsum[:], rhs=ones128[:],
                             start=True, stop=True)
            nsb = cpool.tile([1, 1], dt.float32, tag="nsb")
            nc.vector.tensor_copy(nsb[:], nps[0:1, 0:1])
            nbc_ps = pes.tile([P, 4], dt.float32, tag="es")
            nc.tensor.matmul(nbc_ps[:, 0:1], lhsT=ones1f[:], rhs=nsb[:],
                             start=True, stop=True)
            nbc = cpool.tile([P, 1], dt.float32, tag="nbc")
            nc.vector.tensor_copy(nbc[:], nbc_ps[:, 0:1])

            denom = cpool.tile([P, 1], dt.float32, tag="denom")
            nc.vector.tensor_scalar_add(denom[:], nbc[:], float(K) * EPS)
            rden = cpool.tile([P, 1], dt.float32, tag="rden")
            nc.vector.reciprocal(rden[:], denom[:])
            factor = cpool.tile([P, 1], dt.float32, tag="factor")
            nc.vector.tensor_mul(out=factor[:], in0=rden[:], in1=nbc[:])
            csm = cpool.tile([P, MCH], dt.float32, tag="csm")
            nc.vector.tensor_scalar_add(csm[:], ncs[:], EPS)
            cs2 = cpool.tile([P, MCH], dt.float32, tag="cs2")
            nc.vector.tensor_scalar(cs2[:], csm[:], factor[:], None,
                                    mybir.AluOpType.mult)
            invcs = cpool.tile([P, MCH], dt.float32, tag="invcs")
            nc.vector.reciprocal(invcs[:], cs2[:])

            # EMA per code-chunk, pipelining compute with output DMAs:
            # es_acc <- 0.01*es_acc ; ea_sb (pre-scaled 0.99) += es_acc = nea
            # ne = nea * (1/cs), reusing es_acc as scratch
            nea_dst = nea_o[:].rearrange("(m p) d -> p m d", p=P)
            ne_dst = ne_o[:].rearrange("(m p) d -> p m d", p=P)
            for m in range(MCH):
                nc.vector.tensor_scalar_mul(es_acc[:, m, :], es_acc[:, m, :],
                                            1.0 - DECAY)
                nc.vector.tensor_add(out=ea_sb[:, m, :], in0=ea_sb[:, m, :],
                                     in1=es_acc[:, m, :])
                nc.sync.dma_start(nea_dst[:, m, :], ea_sb[:, m, :])
                nc.vector.tensor_scalar(es_acc[:, m, :], ea_sb[:, m, :],
                                        invcs[:, m:m + 1], None,
                                        mybir.AluOpType.mult)
                nc.sync.dma_start(ne_dst[:, m, :], es_acc[:, m, :])

    nc.compile()
    return nc


def _get_nc(mode=None):
    mode = mode or SCORES_MODE
    if mode not in _CACHE:
        _CACHE[mode] = _build(mode)
    return _CACHE[mode]


def _split_hi_lo_f32(v64, mask):
    hi = v64.astype(np.float32)
    hi = (hi.view(np.uint32) & mask).view(np.float32)
    lo = (v64 - hi.astype(np.float64)).astype(np.float32)
    return hi, lo


def _prep_inputs(x, embed, cluster_size, embed_avg, mode):
    xf = np.ascontiguousarray(np.asarray(x, dtype=np.float32).reshape(N, D))
    embed = np.ascontiguousarray(np.asarray(embed, dtype=np.float32))
    cluster_size = np.ascontiguousarray(
        np.asarray(cluster_size, dtype=np.float32))
    embed_avg = np.ascontiguousarray(np.asarray(embed_avg, dtype=np.float32))

    q64 = (embed.astype(np.float64) ** 2).sum(axis=1)
    b64 = -q64 / 2.0
    if mode == "fp32r":
        bh, bl = _split_hi_lo_f32(b64, BIAS_HI_MASK)
        bias2 = np.stack([bh, bl]).astype(np.float32)
        embT = np.ascontiguousarray(embed.T)
    else:
        q2rep = np.ascontiguousarray(
            np.tile((q64 / 2.0).astype(np.float32), (P, 1)))
        eT = np.ascontiguousarray(embed.T)
        embTh = eT.astype(np.float16)
        embTl = (eT - embTh.astype(np.float32)).astype(np.float16)

    in_maps = []
    for r in range(N_CORES):
        sl = slice(r * NLOC, (r + 1) * NLOC)
        xr = xf[sl]
        m = {
            "x16": xr.astype(np.float16),
            "embed": embed,
            "cs_in": cluster_size,
            "ea_in": embed_avg,
        }
        xrT = np.ascontiguousarray(xr.T)
        if mode == "fp32r":
            m["xT"] = xrT
            m["embT"] = embT
            m["bias2"] = bias2
        else:
            xh = xrT.astype(np.float16)
            m["xTh"] = xh
            m["xTl"] = (xrT - xh.astype(np.float32)).astype(np.float16)
            m["embTh"] = embTh
            m["embTl"] = embTl
            m["q2rep"] = q2rep
        in_maps.append(m)
    return in_maps


def _assemble(results):
    quant = np.concatenate([r["quant"] for r in results], axis=0)
    quantize = quant.reshape(B0, B1, D)
    ind = np.concatenate([r["ind_o"] for r in results]).view(np.int32)
    embed_ind = ind.reshape(B0, B1)
    r0 = results[0]
    ncs = r0["ncs_o"]
    nea = r0["nea_o"]
    ne = r0["ne_o"]
    return quantize, embed_ind, ncs, nea, ne


def _run_full(inputs, mode=None, trace=False, **run_kwargs):
    mode = mode or SCORES_MODE
    nc = _get_nc(mode)
    in_maps = _prep_inputs(inputs["x"], inputs["embed"],
                           inputs["cluster_size"], inputs["embed_avg"], mode)
    res = bass_utils.run_bass_kernel_spmd(
        nc, in_maps, core_ids=list(range(N_CORES)), trace=trace, **run_kwargs)
    return _assemble(res.results), res


def kernel(x, embed, cluster_size, embed_avg):
    out, _ = _run_full(dict(x=x, embed=embed, cluster_size=cluster_size,
                            embed_avg=embed_avg))
    return out
